# revision 2
# baseline (speedup 1.0000x reference)
"""Trainium2 Bass kernel for the NMS-detection KD loss (nn_BaseDefender).

Semantics (mirrors the reference):
    iou[i,j]  = I/(S+T-I) over student/teacher boxes (pixel +1 convention)
    max/argmax over teachers per student row, threshold 0.75
    above_term = sum(above * KL(teacher[argmax] || exposed via log-student)) / (n_above*C)
    below_term = sum(below * -log ps[:, 90]) / (n_below*C)
    out = above_term + below_term  (f32 scalar)

Device strategy: shard student rows across 8 NeuronCores (1024 rows each).
Per core, for each 128-row student tile, compute s[i,j] = ln(I) - ln(S+T) over
all 8192 teachers (monotone in iou: iou = r/(1-r), r = I/(S+T); threshold
iou > 0.75 <=> s > ln(3/7)).  Row max + argmax via the DVE top-8 max/max_index
instructions.  Teacher preds gathered by argmax via indirect DMA; per-row KL
computed on-device with a fused multiply-accumulate.  Host sums the per-row
partials (8 x [1024, 4] outputs) into the final scalar.
"""

import sys

sys.path.insert(0, "/opt/trn_rl_repo")

import numpy as np

NS, NT, C = 8192, 8192, 91
NCORES = 8
SR = NS // NCORES  # student rows per core
P = 128
STILES = SR // P  # student tiles per core
NQ = 4  # teacher quarters
QW = NT // NQ  # teacher quarter width
THRESHOLD = 0.75
NO_OBJECT_INDEX = 90
LOGTHR = float(np.float32(np.log(THRESHOLD / (1.0 + THRESHOLD))))  # ln(3/7)
EPS = 1e-12

_CACHE = {}


def _build_program():
    import concourse.bacc as bacc
    import concourse.bass as bass
    import concourse.mybir as mybir
    import concourse.tile as tile

    f32 = mybir.dt.float32
    u32 = mybir.dt.uint32
    Alu = mybir.AluOpType
    Act = mybir.ActivationFunctionType

    nc = bacc.Bacc("TRN2", target_bir_lowering=False, debug=False, num_devices=NCORES)

    # saug cols: x1, y1, x2+1, y2+1, area, -, -, - (padded to 8)
    saug_d = nc.declare_dram_parameter("saug", [SR, 8], f32, isOutput=False)
    # taug rows: x1t, y1t, x2t+1, y2t+1, areaT
    taug_d = nc.declare_dram_parameter("taug", [5, NT], f32, isOutput=False)
    ps_d = nc.declare_dram_parameter("ps", [SR, C], f32, isOutput=False)
    pt_d = nc.declare_dram_parameter("pt", [NT, C], f32, isOutput=False)
    # out cols: above, kl_row, -log ps[:,90], argmax index (f32)
    out_d = nc.declare_dram_parameter("partials", [SR, 4], f32, isOutput=True)

    def pbcast(dram_row_ap, n_part):
        # DRAM AP replicated across n_part partitions (step-0 partition dim)
        return bass.AP(
            tensor=dram_row_ap.tensor,
            offset=dram_row_ap.offset,
            ap=[[0, n_part]] + list(dram_row_ap.ap),
        )

    with tile.TileContext(nc) as tc:
        with (
            tc.tile_pool(name="bc", bufs=2) as bc_pool,
            tc.tile_pool(name="tmp", bufs=2) as tmp,
            tc.tile_pool(name="spool", bufs=2) as spool,
            tc.tile_pool(name="singles", bufs=1) as singles,
            tc.tile_pool(name="kl", bufs=2) as klp,
        ):
            # student per-tile scalars, loaded once
            sa = []
            for st in range(STILES):
                t = singles.tile([P, 8], f32, tag=f"sa{st}", name=f"sa{st}")
                nc.sync.dma_start(t[:], saug_d[st * P : (st + 1) * P, :])
                sa.append(t)

            # per (stile, quarter) stashes of top-8 values / indices
            m8s = [
                [
                    singles.tile([P, 8], f32, tag=f"m8_{st}_{q}", name=f"m8_{st}_{q}")
                    for q in range(NQ)
                ]
                for st in range(STILES)
            ]
            i8s = [
                [
                    singles.tile([P, 8], u32, tag=f"i8_{st}_{q}", name=f"i8_{st}_{q}")
                    for q in range(NQ)
                ]
                for st in range(STILES)
            ]

            for q in range(NQ):
                qs = q * QW
                x1t = bc_pool.tile([P, QW], f32, tag="x1t")
                y1t = bc_pool.tile([P, QW], f32, tag="y1t")
                x2t = bc_pool.tile([P, QW], f32, tag="x2t")
                y2t = bc_pool.tile([P, QW], f32, tag="y2t")
                art = bc_pool.tile([P, QW], f32, tag="art")
                for t, row in ((x1t, 0), (y1t, 1), (x2t, 2), (y2t, 3), (art, 4)):
                    nc.sync.dma_start(t[:], pbcast(taug_d[row, qs : qs + QW], P))

                for st in range(STILES):
                    x1s = sa[st][:, 0:1]
                    y1s = sa[st][:, 1:2]
                    x2s = sa[st][:, 2:3]
                    y2s = sa[st][:, 3:4]
                    ars = sa[st][:, 4:5]

                    t0 = tmp.tile([P, QW], f32, tag="t0")
                    # t0 = max(x1t, x1s)
                    nc.vector.tensor_scalar(t0[:], x1t[:], x1s, None, Alu.max)
                    t1 = tmp.tile([P, QW], f32, tag="t1")
                    # t1 = min(x2t+1, x2s+1) - t0   (= wr)
                    nc.vector.scalar_tensor_tensor(
                        t1[:], x2t[:], x2s, t0[:], Alu.min, Alu.subtract
                    )
                    t0b = tmp.tile([P, QW], f32, tag="t0")
                    # t0b = max(y1t, y1s)
                    nc.vector.tensor_scalar(t0b[:], y1t[:], y1s, None, Alu.max)
                    t2 = tmp.tile([P, QW], f32, tag="t2")
                    # t2 = min(y2t+1, y2s+1) - t0b  (= hr)
                    nc.vector.scalar_tensor_tensor(
                        t2[:], y2t[:], y2s, t0b[:], Alu.min, Alu.subtract
                    )
                    t0c = tmp.tile([P, QW], f32, tag="t0")
                    # t0c = max(hr, EPS)
                    nc.vector.tensor_scalar(t0c[:], t2[:], EPS, None, Alu.max)
                    t2b = tmp.tile([P, QW], f32, tag="t2")
                    # t2b = max(wr, EPS) * t0c      (= inter, > 0)
                    nc.vector.scalar_tensor_tensor(
                        t2b[:], t1[:], EPS, t0c[:], Alu.max, Alu.mult
                    )
                    t1b = tmp.tile([P, QW], f32, tag="t1")
                    # t1b = ln(inter)
                    nc.scalar.activation(t1b[:], t2b[:], Act.Ln)
                    t0d = tmp.tile([P, QW], f32, tag="t0")
                    # t0d = ln(areaT + areaS)
                    nc.scalar.activation(t0d[:], art[:], Act.Ln, bias=ars, scale=1.0)
                    s = spool.tile([P, QW], f32, tag="s")
                    # s = ln(I) - ln(S+T)
                    nc.vector.tensor_tensor(s[:], t1b[:], t0d[:], Alu.subtract)

                    nc.vector.max(m8s[st][q][:], s[:])
                    nc.vector.max_index(i8s[st][q][:], m8s[st][q][:], s[:])

            # merge quarters, gather, KL, write partials
            for st in range(STILES):
                bv = klp.tile([P, 1], f32, tag="bv")
                bi = klp.tile([P, 1], u32, tag="bi")
                nc.vector.tensor_copy(bv[:], m8s[st][0][:, 0:1])
                nc.vector.tensor_copy(bi[:], i8s[st][0][:, 0:1])
                for q in range(1, NQ):
                    win = klp.tile([P, 1], u32, tag="win")
                    nc.vector.tensor_tensor(
                        win[:], m8s[st][q][:, 0:1], bv[:], Alu.is_gt
                    )
                    adj = klp.tile([P, 1], u32, tag="adj")
                    nc.vector.tensor_scalar(
                        adj[:], i8s[st][q][:, 0:1], q * QW, None, Alu.add
                    )
                    nc.vector.copy_predicated(bv[:], win[:], m8s[st][q][:, 0:1])
                    nc.vector.copy_predicated(bi[:], win[:], adj[:])

                stage = klp.tile([P, 4], f32, tag="stage")
                # above flag
                nc.vector.tensor_scalar(
                    stage[:, 0:1], bv[:], LOGTHR, None, Alu.is_gt
                )
                # index as f32 (diagnostic)
                nc.vector.tensor_copy(stage[:, 3:4], bi[:])

                ptg = klp.tile([P, C], f32, tag="ptg")
                nc.gpsimd.indirect_dma_start(
                    out=ptg[:],
                    out_offset=None,
                    in_=pt_d[:],
                    in_offset=bass.IndirectOffsetOnAxis(ap=bi[:, 0:1], axis=0),
                )
                pst = klp.tile([P, C], f32, tag="pst")
                nc.sync.dma_start(pst[:], ps_d[st * P : (st + 1) * P, :])
                lps = klp.tile([P, C], f32, tag="lps")
                nc.scalar.activation(lps[:], pst[:], Act.Ln)
                ptc = klp.tile([P, C], f32, tag="ptc")
                nc.vector.tensor_scalar(ptc[:], ptg[:], 1e-38, None, Alu.max)
                lpt = klp.tile([P, C], f32, tag="lpt")
                nc.scalar.activation(lpt[:], ptc[:], Act.Ln)
                d = klp.tile([P, C], f32, tag="d")
                nc.vector.tensor_tensor(d[:], lpt[:], lps[:], Alu.subtract)
                junk = klp.tile([P, C], f32, tag="junk")
                # kl_row = sum(d * pt) fused into accum_out
                nc.vector.scalar_tensor_tensor(
                    junk[:], d[:], 0.0, ptg[:], Alu.add, Alu.mult,
                    accum_out=stage[:, 1:2],
                )
                # -log ps[:, NO_OBJECT]
                nc.vector.tensor_scalar(
                    stage[:, 2:3], lps[:, NO_OBJECT_INDEX : NO_OBJECT_INDEX + 1],
                    -1.0, None, Alu.mult,
                )
                nc.sync.dma_start(out_d[st * P : (st + 1) * P, :], stage[:])

    nc.compile()
    return nc


def _get_program():
    if "nc" not in _CACHE:
        _CACHE["nc"] = _build_program()
    return _CACHE["nc"]


def _prep_inputs(boxes_student, boxes_teacher, pred_student, pred_teacher):
    one = np.float32(1.0)
    bs = np.asarray(boxes_student, dtype=np.float32)
    bt = np.asarray(boxes_teacher, dtype=np.float32)
    ps = np.ascontiguousarray(np.asarray(pred_student, dtype=np.float32))
    pt = np.ascontiguousarray(np.asarray(pred_teacher, dtype=np.float32))

    # all-f32 arithmetic, mirroring the reference's fp32 ops
    saug = np.zeros((NS, 8), dtype=np.float32)
    saug[:, 0] = bs[:, 0]
    saug[:, 1] = bs[:, 1]
    saug[:, 2] = bs[:, 2] + one
    saug[:, 3] = bs[:, 3] + one
    saug[:, 4] = (bs[:, 2] - bs[:, 0] + one) * (bs[:, 3] - bs[:, 1] + one)

    taug = np.zeros((5, NT), dtype=np.float32)
    taug[0] = bt[:, 0]
    taug[1] = bt[:, 1]
    taug[2] = bt[:, 2] + one
    taug[3] = bt[:, 3] + one
    taug[4] = (bt[:, 2] - bt[:, 0] + one) * (bt[:, 3] - bt[:, 1] + one)
    taug = np.ascontiguousarray(taug)

    in_maps = []
    for i in range(NCORES):
        sl = slice(i * SR, (i + 1) * SR)
        in_maps.append(
            {
                "saug": np.ascontiguousarray(saug[sl]),
                "taug": taug,
                "ps": np.ascontiguousarray(ps[sl]),
                "pt": pt,
            }
        )
    return in_maps


def _finish(parts):
    parts = parts.astype(np.float64)
    above = parts[:, 0]
    kl = parts[:, 1]
    m90 = parts[:, 2]
    n_above = above.sum()
    n_below = NS - n_above
    above_term = (above * kl).sum() / (n_above * C) if n_above > 0 else 0.0
    below_term = ((1.0 - above) * m90).sum() / (n_below * C) if n_below > 0 else 0.0
    return np.float32(above_term + below_term)


def kernel(boxes_student, boxes_teacher, pred_student, pred_teacher, _trace=False):
    from concourse.bass_utils import run_bass_kernel_spmd

    nc = _get_program()
    in_maps = _prep_inputs(boxes_student, boxes_teacher, pred_student, pred_teacher)
    res = run_bass_kernel_spmd(nc, in_maps, list(range(NCORES)), trace=_trace)
    _CACHE["last_results"] = res
    parts = np.concatenate([res.results[i]["partials"] for i in range(NCORES)], axis=0)
    _CACHE["last_parts"] = parts
    return _finish(parts)


if __name__ == "__main__":
    rng = np.random.default_rng(0)
    xy = rng.random((NS, 2), dtype=np.float32) * 1000
    wh = rng.random((NS, 2), dtype=np.float32) * 100 + 4
    bs = np.concatenate([xy, xy + wh], 1)
    xy = rng.random((NT, 2), dtype=np.float32) * 1000
    wh = rng.random((NT, 2), dtype=np.float32) * 100 + 4
    bt = np.concatenate([xy, xy + wh], 1)
    ps = rng.random((NS, C), dtype=np.float32) + 0.01
    ps /= ps.sum(1, keepdims=True)
    pt = rng.random((NT, C), dtype=np.float32) + 0.01
    pt /= pt.sum(1, keepdims=True)
    print("out:", kernel(bs, bt, ps, pt))


# revision 3
# speedup vs baseline: 1.1753x; 1.1753x over previous
"""Trainium2 Bass kernel for the NMS-detection KD loss (nn_BaseDefender).

Semantics (mirrors the reference):
    iou[i,j]  = I/(S+T-I) over student/teacher boxes (pixel +1 convention)
    max/argmax over teachers per student row, threshold 0.75
    above_term = sum(above * KL(pt[argmax] || ps)) / (n_above*C)
    below_term = sum(below * -log ps[:, 90]) / (n_below*C)
    out = above_term + below_term  (f32 scalar)

Device strategy: students sharded across 8 NeuronCores (1024 rows each), both
box sets sorted by x1 on the host.  A student tile of 128 sorted rows can only
x-overlap a contiguous window of sorted teachers, so each tile computes
s[i,j] = ln(I) - ln(S+T) (monotone in iou; iou>0.75 <=> s > ln(3/7);
ln(0) = -inf for non-overlaps) over its assigned window columns only.  Row
max/argmax via the DVE top-8 max/max_index instructions, teacher preds
gathered by argmax via indirect DMA, per-row KL reduced on-device with a
fused multiply-accumulate.  Host sums per-row partials into the scalar.
"""

import sys

sys.path.insert(0, "/opt/trn_rl_repo")

import numpy as np

NS, NT, C = 8192, 8192, 91
NCORES = 8
SR = NS // NCORES  # student rows per core
P = 128
STILES = SR // P  # student tiles per core
THRESHOLD = 0.75
NO_OBJECT_INDEX = 90
LOGTHR = float(np.float32(np.log(THRESHOLD / (1.0 + THRESHOLD))))  # ln(3/7)
PAD = 4096  # teacher padding on each side of the sorted array

_CACHE = {}


def _build_program(W, R, A):
    """W: per-core teacher window width; per stile k the computed columns are
    [A*k, A*k + R) of the window."""
    import concourse.bacc as bacc
    import concourse.bass as bass
    import concourse.mybir as mybir
    import concourse.tile as tile

    f32 = mybir.dt.float32
    u32 = mybir.dt.uint32
    Alu = mybir.AluOpType
    Act = mybir.ActivationFunctionType

    nc = bacc.Bacc("TRN2", target_bir_lowering=False, debug=False, num_devices=NCORES)

    # saug cols: x1, y1, x2+1, y2+1, area, -, -, - (padded to 8)
    saug_d = nc.declare_dram_parameter("saug", [SR, 8], f32, isOutput=False)
    # taug rows: x1t, y1t, x2t+1, y2t+1, areaT over this core's window
    taug_d = nc.declare_dram_parameter("taug", [5, W], f32, isOutput=False)
    ps_d = nc.declare_dram_parameter("ps", [SR, C], f32, isOutput=False)
    pt_d = nc.declare_dram_parameter("pt", [W, C], f32, isOutput=False)
    # out cols: above, kl_row, -log ps[:,90], window argmax index (f32)
    out_d = nc.declare_dram_parameter("partials", [SR, 4], f32, isOutput=True)

    def pbcast(dram_row_ap, n_part):
        return bass.AP(
            tensor=dram_row_ap.tensor,
            offset=dram_row_ap.offset,
            ap=[[0, n_part]] + list(dram_row_ap.ap),
        )

    with tile.TileContext(nc) as tc:
        with (
            tc.tile_pool(name="bc", bufs=1) as bc_pool,
            tc.tile_pool(name="tmp", bufs=2) as tmp,
            tc.tile_pool(name="spool", bufs=2) as spool,
            tc.tile_pool(name="singles", bufs=1) as singles,
            tc.tile_pool(name="kl", bufs=2) as klp,
        ):
            x1t = bc_pool.tile([P, W], f32, tag="x1t")
            y1t = bc_pool.tile([P, W], f32, tag="y1t")
            x2t = bc_pool.tile([P, W], f32, tag="x2t")
            y2t = bc_pool.tile([P, W], f32, tag="y2t")
            art = bc_pool.tile([P, W], f32, tag="art")
            for t, row in ((x1t, 0), (x2t, 2), (y1t, 1), (y2t, 3), (art, 4)):
                nc.sync.dma_start(t[:], pbcast(taug_d[row, :], P))

            sa = []
            for st in range(STILES):
                t = singles.tile([P, 8], f32, tag=f"sa{st}", name=f"sa{st}")
                nc.sync.dma_start(t[:], saug_d[st * P : (st + 1) * P, :])
                sa.append(t)

            m8s = [
                singles.tile([P, 8], f32, tag=f"m8_{st}", name=f"m8_{st}")
                for st in range(STILES)
            ]
            i8s = [
                singles.tile([P, 8], u32, tag=f"i8_{st}", name=f"i8_{st}")
                for st in range(STILES)
            ]

            for st in range(STILES):
                x1s = sa[st][:, 0:1]
                y1s = sa[st][:, 1:2]
                x2s = sa[st][:, 2:3]
                y2s = sa[st][:, 3:4]
                ars = sa[st][:, 4:5]
                lo = A * st
                cols = slice(lo, lo + R)

                t0 = tmp.tile([P, R], f32, tag="t0")
                # t0 = max(x1t, x1s)
                nc.vector.tensor_scalar(t0[:], x1t[:, cols], x1s, None, Alu.max)
                t1 = tmp.tile([P, R], f32, tag="t1")
                # t1 = min(x2t+1, x2s+1) - t0          (wr)
                nc.vector.scalar_tensor_tensor(
                    t1[:], x2t[:, cols], x2s, t0[:], Alu.min, Alu.subtract
                )
                t0b = tmp.tile([P, R], f32, tag="t0")
                # t0b = max(y1t, y1s)
                nc.vector.tensor_scalar(t0b[:], y1t[:, cols], y1s, None, Alu.max)
                t2 = tmp.tile([P, R], f32, tag="t2")
                # t2 = min(y2t+1, y2s+1) - t0b         (hr)
                nc.vector.scalar_tensor_tensor(
                    t2[:], y2t[:, cols], y2s, t0b[:], Alu.min, Alu.subtract
                )
                t0c = tmp.tile([P, R], f32, tag="t0")
                # t0c = relu(hr)   (on ACT)
                nc.scalar.activation(t0c[:], t2[:], Act.Relu)
                t2b = tmp.tile([P, R], f32, tag="t2")
                # t2b = relu(wr) * t0c                 (inter, exact; 0 when disjoint)
                nc.vector.scalar_tensor_tensor(
                    t2b[:], t1[:], 0.0, t0c[:], Alu.max, Alu.mult
                )
                t1b = tmp.tile([P, R], f32, tag="t1")
                # t1b = ln(inter)        (-inf for non-overlap)
                nc.scalar.activation(t1b[:], t2b[:], Act.Ln)
                t0d = tmp.tile([P, R], f32, tag="t0")
                # t0d = ln(areaT + areaS)
                nc.scalar.activation(t0d[:], art[:, cols], Act.Ln, bias=ars, scale=1.0)
                s = spool.tile([P, R], f32, tag="s")
                nc.vector.tensor_tensor(s[:], t1b[:], t0d[:], Alu.subtract)

                nc.vector.max(m8s[st][:], s[:])
                nc.vector.max_index(i8s[st][:], m8s[st][:], s[:])

            for st in range(STILES):
                bv = m8s[st][:, 0:1]
                bi = klp.tile([P, 1], u32, tag="bi")
                # window-global argmax index
                nc.vector.tensor_scalar(
                    bi[:], i8s[st][:, 0:1], A * st, None, Alu.add
                )

                stage = klp.tile([P, 4], f32, tag="stage")
                nc.vector.tensor_scalar(stage[:, 0:1], bv, LOGTHR, None, Alu.is_gt)
                nc.vector.tensor_copy(stage[:, 3:4], bi[:])

                ptg = klp.tile([P, C], f32, tag="ptg")
                nc.gpsimd.indirect_dma_start(
                    out=ptg[:],
                    out_offset=None,
                    in_=pt_d[:],
                    in_offset=bass.IndirectOffsetOnAxis(ap=bi[:, 0:1], axis=0),
                )
                pst = klp.tile([P, C], f32, tag="pst")
                nc.sync.dma_start(pst[:], ps_d[st * P : (st + 1) * P, :])
                lps = klp.tile([P, C], f32, tag="lps")
                nc.scalar.activation(lps[:], pst[:], Act.Ln)
                ptc = klp.tile([P, C], f32, tag="ptc")
                nc.vector.tensor_scalar(ptc[:], ptg[:], 1e-38, None, Alu.max)
                lpt = klp.tile([P, C], f32, tag="lpt")
                nc.scalar.activation(lpt[:], ptc[:], Act.Ln)
                d = klp.tile([P, C], f32, tag="d")
                nc.vector.tensor_tensor(d[:], lpt[:], lps[:], Alu.subtract)
                junk = klp.tile([P, C], f32, tag="junk")
                nc.vector.scalar_tensor_tensor(
                    junk[:], d[:], 0.0, ptg[:], Alu.add, Alu.mult,
                    accum_out=stage[:, 1:2],
                )
                nc.vector.tensor_scalar(
                    stage[:, 2:3], lps[:, NO_OBJECT_INDEX : NO_OBJECT_INDEX + 1],
                    -1.0, None, Alu.mult,
                )
                nc.sync.dma_start(out_d[st * P : (st + 1) * P, :], stage[:])

    nc.compile()
    return nc


def _get_program(W, R, A):
    key = ("nc", W, R, A)
    if key not in _CACHE:
        _CACHE[key] = _build_program(W, R, A)
    return _CACHE[key]


def _plan_windows(saug_sorted, x1t_sorted, x2t_sorted):
    """Choose per-core window starts W0[c] plus shared (A, R, W) so stile k of
    every core covers its exact needed teacher range inside [A*k, A*k+R)."""
    lo = np.zeros((NCORES, STILES), np.int64)
    hi = np.zeros((NCORES, STILES), np.int64)
    max_tw = float(np.max(x2t_sorted - x1t_sorted))  # widest teacher
    for c in range(NCORES):
        for k in range(STILES):
            r0 = c * SR + k * P
            rows_x1 = saug_sorted[r0 : r0 + P, 0]
            rows_x2p1 = saug_sorted[r0 : r0 + P, 2]
            # teacher j can overlap iff x1t_j <= max(x2s)+1-? (x1t < x2s+1)
            # and x2t_j >= min(x1s)-1  =>  x1t_j >= min(x1s)-1-max_tw
            lo[c, k] = np.searchsorted(x1t_sorted, rows_x1.min() - 1.0 - max_tw, "left")
            hi[c, k] = np.searchsorted(x1t_sorted, rows_x2p1.max(), "right")

    best = None
    for A in range(0, 1025, 16):
        W0 = np.min(lo - A * np.arange(STILES)[None, :], axis=1)
        R = int(np.max(hi - (W0[:, None] + A * np.arange(STILES)[None, :])))
        R = max(R, 16)
        R = (R + 511) // 512 * 512
        W = A * (STILES - 1) + R
        if best is None or W < best[0]:
            best = (W, R, A, W0.copy())
    W, R, A, W0 = best
    return W, R, A, W0


def _prep_inputs(boxes_student, boxes_teacher, pred_student, pred_teacher):
    one = np.float32(1.0)
    bs = np.asarray(boxes_student, dtype=np.float32)
    bt = np.asarray(boxes_teacher, dtype=np.float32)
    ps = np.asarray(pred_student, dtype=np.float32)
    pt = np.asarray(pred_teacher, dtype=np.float32)

    sidx = np.argsort(bs[:, 0], kind="stable")
    tidx = np.argsort(bt[:, 0], kind="stable")
    bs_s = bs[sidx]
    bt_s = bt[tidx]

    saug = np.zeros((NS, 8), dtype=np.float32)
    saug[:, 0] = bs_s[:, 0]
    saug[:, 1] = bs_s[:, 1]
    saug[:, 2] = bs_s[:, 2] + one
    saug[:, 3] = bs_s[:, 3] + one
    saug[:, 4] = (bs_s[:, 2] - bs_s[:, 0] + one) * (bs_s[:, 3] - bs_s[:, 1] + one)

    W, R, A, W0 = _plan_windows(saug, bt_s[:, 0], bt_s[:, 2])

    # padded sorted teacher arrays
    FAR = np.float32(3e6)
    taug_all = np.full((5, NT + 2 * PAD), FAR, dtype=np.float32)
    taug_all[2, :] = FAR + 11.0
    taug_all[3, :] = FAR + 11.0
    taug_all[4, :] = np.float32(121.0)
    taug_all[0, PAD : PAD + NT] = bt_s[:, 0]
    taug_all[1, PAD : PAD + NT] = bt_s[:, 1]
    taug_all[2, PAD : PAD + NT] = bt_s[:, 2] + one
    taug_all[3, PAD : PAD + NT] = bt_s[:, 3] + one
    taug_all[4, PAD : PAD + NT] = (bt_s[:, 2] - bt_s[:, 0] + one) * (
        bt_s[:, 3] - bt_s[:, 1] + one
    )
    pt_all = np.full((NT + 2 * PAD, C), np.float32(1.0 / C), dtype=np.float32)
    pt_all[PAD : PAD + NT] = pt[tidx]
    ps_sorted = ps[sidx]

    assert W0.min() + PAD >= 0 and W0.max() + W <= NT + PAD, (W0.min(), W0.max(), W)

    in_maps = []
    for c in range(NCORES):
        sl = slice(c * SR, (c + 1) * SR)
        w0 = int(W0[c]) + PAD
        in_maps.append(
            {
                "saug": np.ascontiguousarray(saug[sl]),
                "taug": np.ascontiguousarray(taug_all[:, w0 : w0 + W]),
                "ps": np.ascontiguousarray(ps_sorted[sl]),
                "pt": np.ascontiguousarray(pt_all[w0 : w0 + W]),
            }
        )
    _CACHE["last_meta"] = {"sidx": sidx, "tidx": tidx, "W0": W0, "W": W, "R": R, "A": A}
    return in_maps, (W, R, A)


def _finish(parts):
    parts = parts.astype(np.float64)
    above = parts[:, 0]
    kl = parts[:, 1]
    m90 = parts[:, 2]
    n_above = above.sum()
    n_below = NS - n_above
    above_term = (above * kl).sum() / (n_above * C) if n_above > 0 else 0.0
    below_term = ((1.0 - above) * m90).sum() / (n_below * C) if n_below > 0 else 0.0
    return np.float32(above_term + below_term)


def kernel(boxes_student, boxes_teacher, pred_student, pred_teacher, _trace=False):
    from concourse.bass_utils import run_bass_kernel_spmd

    in_maps, (W, R, A) = _prep_inputs(
        boxes_student, boxes_teacher, pred_student, pred_teacher
    )
    nc = _get_program(W, R, A)
    res = run_bass_kernel_spmd(nc, in_maps, list(range(NCORES)), trace=_trace)
    _CACHE["last_results"] = res
    parts = np.concatenate([res.results[i]["partials"] for i in range(NCORES)], axis=0)
    _CACHE["last_parts"] = parts
    return _finish(parts)


if __name__ == "__main__":
    rng = np.random.default_rng(0)
    xy = rng.random((NS, 2), dtype=np.float32) * 1000
    wh = rng.random((NS, 2), dtype=np.float32) * 100 + 4
    bs = np.concatenate([xy, xy + wh], 1)
    xy = rng.random((NT, 2), dtype=np.float32) * 1000
    wh = rng.random((NT, 2), dtype=np.float32) * 100 + 4
    bt = np.concatenate([xy, xy + wh], 1)
    ps = rng.random((NS, C), dtype=np.float32) + 0.01
    ps /= ps.sum(1, keepdims=True)
    pt = rng.random((NT, C), dtype=np.float32) + 0.01
    pt /= pt.sum(1, keepdims=True)
    print("out:", kernel(bs, bt, ps, pt))


# revision 5
# speedup vs baseline: 3.4672x; 2.9500x over previous
"""Trainium2 Bass kernel for the NMS-detection KD loss (nn_BaseDefender).

Semantics (mirrors the reference):
    iou[i,j]  = I/(S+T-I) over student/teacher boxes (pixel +1 convention)
    max/argmax over teachers per student row, threshold 0.75
    above_term = sum(above * KL(pt[argmax] || ps)) / (n_above*C)
    below_term = sum(below * -log ps[:, 90]) / (n_below*C)
    out = above_term + below_term  (f32 scalar)

Device strategy: students sharded across 8 NeuronCores (1024 rows each), both
box sets sorted by x1 on the host.  A student tile of 128 sorted rows can only
x-overlap a contiguous window of sorted teachers, so each tile computes
s[i,j] = ln(I) - ln(S+T) (monotone in iou; iou>0.75 <=> s > ln(3/7);
ln(0) = -inf for non-overlaps) over its assigned window columns only.  Row
max/argmax via the DVE top-8 max/max_index instructions, teacher preds
gathered by argmax via indirect DMA, per-row KL reduced on-device with a
fused multiply-accumulate.  Host sums per-row partials into the scalar.
"""

import sys

sys.path.insert(0, "/opt/trn_rl_repo")

import numpy as np

NS, NT, C = 8192, 8192, 91
NCORES = 8
SR = NS // NCORES  # student rows per core
P = 128
STILES = SR // P  # student tiles per core
THRESHOLD = 0.75
NO_OBJECT_INDEX = 90
LOGTHR = float(np.float32(np.log(THRESHOLD / (1.0 + THRESHOLD))))  # ln(3/7)
PAD = 4096  # teacher padding on each side of the sorted array

_CACHE = {}


def _build_program(W, R, A, reps=1):
    """W: per-core teacher window width; per stile k the computed columns are
    [A*k, A*k + R) of the window.  reps>1 replicates the body (benchmarking)."""
    import concourse.bacc as bacc
    import concourse.bass as bass
    import concourse.mybir as mybir
    import concourse.tile as tile

    f32 = mybir.dt.float32
    u32 = mybir.dt.uint32
    Alu = mybir.AluOpType
    Act = mybir.ActivationFunctionType

    nc = bacc.Bacc("TRN2", target_bir_lowering=False, debug=False, num_devices=NCORES)

    # saug cols: x1, y1, x2+1, y2+1, area, -, -, - (padded to 8)
    saug_d = nc.declare_dram_parameter("saug", [SR, 8], f32, isOutput=False)
    # taug rows: x1t, y1t, x2t+1, y2t+1, areaT over this core's window
    taug_d = nc.declare_dram_parameter("taug", [5, W], f32, isOutput=False)
    ps_d = nc.declare_dram_parameter("ps", [SR, C], f32, isOutput=False)
    pt_d = nc.declare_dram_parameter("pt", [W, C], f32, isOutput=False)
    # out cols: above, kl_row, -log ps[:,90], window argmax index (f32)
    out_d = nc.declare_dram_parameter("partials", [SR, 4], f32, isOutput=True)

    def pbcast(dram_row_ap, n_part):
        return bass.AP(
            tensor=dram_row_ap.tensor,
            offset=dram_row_ap.offset,
            ap=[[0, n_part]] + list(dram_row_ap.ap),
        )

    with tile.TileContext(nc) as tc:
        with (
            tc.tile_pool(name="bc", bufs=1) as bc_pool,
            tc.tile_pool(name="tmp", bufs=2) as tmp,
            tc.tile_pool(name="spool", bufs=2) as spool,
            tc.tile_pool(name="singles", bufs=1) as singles,
            tc.tile_pool(name="kl", bufs=2) as klp,
        ):
          for rep in range(reps):
            x1t = bc_pool.tile([P, W], f32, tag="x1t")
            y1t = bc_pool.tile([P, W], f32, tag="y1t")
            x2t = bc_pool.tile([P, W], f32, tag="x2t")
            y2t = bc_pool.tile([P, W], f32, tag="y2t")
            art = bc_pool.tile([P, W], f32, tag="art")
            for t, row in ((x1t, 0), (x2t, 2), (y1t, 1), (y2t, 3), (art, 4)):
                nc.sync.dma_start(t[:], pbcast(taug_d[row, :], P))

            sa = []
            for st in range(STILES):
                t = singles.tile([P, 8], f32, tag=f"sa{st}", name=f"sa{st}")
                nc.sync.dma_start(t[:], saug_d[st * P : (st + 1) * P, :])
                sa.append(t)

            m8s = [
                singles.tile([P, 8], f32, tag=f"m8_{st}", name=f"m8_{st}")
                for st in range(STILES)
            ]
            i8s = [
                singles.tile([P, 8], u32, tag=f"i8_{st}", name=f"i8_{st}")
                for st in range(STILES)
            ]

            for st in range(STILES):
                x1s = sa[st][:, 0:1]
                y1s = sa[st][:, 1:2]
                x2s = sa[st][:, 2:3]
                y2s = sa[st][:, 3:4]
                ars = sa[st][:, 4:5]
                lo = A * st
                cols = slice(lo, lo + R)

                t0 = tmp.tile([P, R], f32, tag="t0")
                # t0 = max(x1t, x1s)
                nc.vector.tensor_scalar(t0[:], x1t[:, cols], x1s, None, Alu.max)
                t1 = tmp.tile([P, R], f32, tag="t1")
                # t1 = min(x2t+1, x2s+1) - t0          (wr)
                nc.vector.scalar_tensor_tensor(
                    t1[:], x2t[:, cols], x2s, t0[:], Alu.min, Alu.subtract
                )
                t0b = tmp.tile([P, R], f32, tag="t0")
                # t0b = max(y1t, y1s)
                nc.vector.tensor_scalar(t0b[:], y1t[:, cols], y1s, None, Alu.max)
                t2 = tmp.tile([P, R], f32, tag="t2")
                # t2 = min(y2t+1, y2s+1) - t0b         (hr)
                nc.vector.scalar_tensor_tensor(
                    t2[:], y2t[:, cols], y2s, t0b[:], Alu.min, Alu.subtract
                )
                t0c = tmp.tile([P, R], f32, tag="t0")
                # t0c = relu(hr)   (on ACT)
                nc.scalar.activation(t0c[:], t2[:], Act.Relu)
                t2b = tmp.tile([P, R], f32, tag="t2")
                # t2b = relu(wr) * t0c                 (inter, exact; 0 when disjoint)
                nc.vector.scalar_tensor_tensor(
                    t2b[:], t1[:], 0.0, t0c[:], Alu.max, Alu.mult
                )
                t1b = tmp.tile([P, R], f32, tag="t1")
                # t1b = ln(inter)        (-inf for non-overlap)
                nc.scalar.activation(t1b[:], t2b[:], Act.Ln)
                t0d = tmp.tile([P, R], f32, tag="t0")
                # t0d = ln(areaT + areaS)
                nc.scalar.activation(t0d[:], art[:, cols], Act.Ln, bias=ars, scale=1.0)
                s = spool.tile([P, R], f32, tag="s")
                nc.vector.tensor_tensor(s[:], t1b[:], t0d[:], Alu.subtract)

                nc.vector.max(m8s[st][:], s[:])
                nc.vector.max_index(i8s[st][:], m8s[st][:], s[:])

            for st in range(STILES):
                bv = m8s[st][:, 0:1]
                bi = klp.tile([P, 1], u32, tag="bi")
                # window-global argmax index
                nc.vector.tensor_scalar(
                    bi[:], i8s[st][:, 0:1], A * st, None, Alu.add
                )

                stage = klp.tile([P, 4], f32, tag="stage")
                nc.vector.tensor_scalar(stage[:, 0:1], bv, LOGTHR, None, Alu.is_gt)
                nc.vector.tensor_copy(stage[:, 3:4], bi[:])

                ptg = klp.tile([P, C], f32, tag="ptg")
                nc.gpsimd.indirect_dma_start(
                    out=ptg[:],
                    out_offset=None,
                    in_=pt_d[:],
                    in_offset=bass.IndirectOffsetOnAxis(ap=bi[:, 0:1], axis=0),
                )
                pst = klp.tile([P, C], f32, tag="pst")
                nc.sync.dma_start(pst[:], ps_d[st * P : (st + 1) * P, :])
                lps = klp.tile([P, C], f32, tag="lps")
                nc.scalar.activation(lps[:], pst[:], Act.Ln)
                ptc = klp.tile([P, C], f32, tag="ptc")
                nc.vector.tensor_scalar(ptc[:], ptg[:], 1e-38, None, Alu.max)
                lpt = klp.tile([P, C], f32, tag="lpt")
                nc.scalar.activation(lpt[:], ptc[:], Act.Ln)
                d = klp.tile([P, C], f32, tag="d")
                nc.vector.tensor_tensor(d[:], lpt[:], lps[:], Alu.subtract)
                junk = klp.tile([P, C], f32, tag="junk")
                nc.vector.scalar_tensor_tensor(
                    junk[:], d[:], 0.0, ptg[:], Alu.add, Alu.mult,
                    accum_out=stage[:, 1:2],
                )
                nc.vector.tensor_scalar(
                    stage[:, 2:3], lps[:, NO_OBJECT_INDEX : NO_OBJECT_INDEX + 1],
                    -1.0, None, Alu.mult,
                )
                nc.sync.dma_start(out_d[st * P : (st + 1) * P, :], stage[:])

    nc.compile()
    return nc


def _get_program(W, R, A):
    key = ("nc", W, R, A)
    if key not in _CACHE:
        _CACHE[key] = _build_program(W, R, A)
    return _CACHE[key]


def _plan_windows(saug_sorted, x1t_sorted, x2t_sorted):
    """Choose per-core window starts W0[c] plus shared (A, R, W) so stile k of
    every core covers its exact needed teacher range inside [A*k, A*k+R)."""
    lo = np.zeros((NCORES, STILES), np.int64)
    hi = np.zeros((NCORES, STILES), np.int64)
    max_tw = float(np.max(x2t_sorted - x1t_sorted))  # widest teacher
    for c in range(NCORES):
        for k in range(STILES):
            r0 = c * SR + k * P
            rows_x1 = saug_sorted[r0 : r0 + P, 0]
            rows_x2p1 = saug_sorted[r0 : r0 + P, 2]
            # teacher j can overlap iff x1t_j <= max(x2s)+1-? (x1t < x2s+1)
            # and x2t_j >= min(x1s)-1  =>  x1t_j >= min(x1s)-1-max_tw
            lo[c, k] = np.searchsorted(x1t_sorted, rows_x1.min() - 1.0 - max_tw, "left")
            hi[c, k] = np.searchsorted(x1t_sorted, rows_x2p1.max(), "right")

    best = None
    for A in range(0, 1025, 16):
        W0 = np.min(lo - A * np.arange(STILES)[None, :], axis=1)
        R = int(np.max(hi - (W0[:, None] + A * np.arange(STILES)[None, :])))
        R = max(R, 16)
        R = (R + 511) // 512 * 512
        W = A * (STILES - 1) + R
        if best is None or W < best[0]:
            best = (W, R, A, W0.copy())
    W, R, A, W0 = best
    return W, R, A, W0


def _prep_inputs(boxes_student, boxes_teacher, pred_student, pred_teacher):
    one = np.float32(1.0)
    bs = np.asarray(boxes_student, dtype=np.float32)
    bt = np.asarray(boxes_teacher, dtype=np.float32)
    ps = np.asarray(pred_student, dtype=np.float32)
    pt = np.asarray(pred_teacher, dtype=np.float32)

    sidx = np.argsort(bs[:, 0], kind="stable")
    tidx = np.argsort(bt[:, 0], kind="stable")
    bs_s = bs[sidx]
    bt_s = bt[tidx]

    saug = np.zeros((NS, 8), dtype=np.float32)
    saug[:, 0] = bs_s[:, 0]
    saug[:, 1] = bs_s[:, 1]
    saug[:, 2] = bs_s[:, 2] + one
    saug[:, 3] = bs_s[:, 3] + one
    saug[:, 4] = (bs_s[:, 2] - bs_s[:, 0] + one) * (bs_s[:, 3] - bs_s[:, 1] + one)

    W, R, A, W0 = _plan_windows(saug, bt_s[:, 0], bt_s[:, 2])

    # padded sorted teacher arrays
    FAR = np.float32(3e6)
    taug_all = np.full((5, NT + 2 * PAD), FAR, dtype=np.float32)
    taug_all[2, :] = FAR + 11.0
    taug_all[3, :] = FAR + 11.0
    taug_all[4, :] = np.float32(121.0)
    taug_all[0, PAD : PAD + NT] = bt_s[:, 0]
    taug_all[1, PAD : PAD + NT] = bt_s[:, 1]
    taug_all[2, PAD : PAD + NT] = bt_s[:, 2] + one
    taug_all[3, PAD : PAD + NT] = bt_s[:, 3] + one
    taug_all[4, PAD : PAD + NT] = (bt_s[:, 2] - bt_s[:, 0] + one) * (
        bt_s[:, 3] - bt_s[:, 1] + one
    )
    pt_all = np.full((NT + 2 * PAD, C), np.float32(1.0 / C), dtype=np.float32)
    pt_all[PAD : PAD + NT] = pt[tidx]
    ps_sorted = ps[sidx]

    assert W0.min() + PAD >= 0 and W0.max() + W <= NT + PAD, (W0.min(), W0.max(), W)

    in_maps = []
    for c in range(NCORES):
        sl = slice(c * SR, (c + 1) * SR)
        w0 = int(W0[c]) + PAD
        in_maps.append(
            {
                "saug": np.ascontiguousarray(saug[sl]),
                "taug": np.ascontiguousarray(taug_all[:, w0 : w0 + W]),
                "ps": np.ascontiguousarray(ps_sorted[sl]),
                "pt": np.ascontiguousarray(pt_all[w0 : w0 + W]),
            }
        )
    _CACHE["last_meta"] = {"sidx": sidx, "tidx": tidx, "W0": W0, "W": W, "R": R, "A": A}
    return in_maps, (W, R, A)


def _finish(parts):
    parts = parts.astype(np.float64)
    above = parts[:, 0]
    kl = parts[:, 1]
    m90 = parts[:, 2]
    n_above = above.sum()
    n_below = NS - n_above
    above_term = (above * kl).sum() / (n_above * C) if n_above > 0 else 0.0
    below_term = ((1.0 - above) * m90).sum() / (n_below * C) if n_below > 0 else 0.0
    return np.float32(above_term + below_term)


def kernel(boxes_student, boxes_teacher, pred_student, pred_teacher, _trace=False):
    from concourse.bass_utils import run_bass_kernel_spmd

    in_maps, (W, R, A) = _prep_inputs(
        boxes_student, boxes_teacher, pred_student, pred_teacher
    )
    nc = _get_program(W, R, A)
    res = run_bass_kernel_spmd(nc, in_maps, list(range(NCORES)), trace=_trace)
    _CACHE["last_results"] = res
    parts = np.concatenate([res.results[i]["partials"] for i in range(NCORES)], axis=0)
    _CACHE["last_parts"] = parts
    return _finish(parts)


if __name__ == "__main__":
    rng = np.random.default_rng(0)
    xy = rng.random((NS, 2), dtype=np.float32) * 1000
    wh = rng.random((NS, 2), dtype=np.float32) * 100 + 4
    bs = np.concatenate([xy, xy + wh], 1)
    xy = rng.random((NT, 2), dtype=np.float32) * 1000
    wh = rng.random((NT, 2), dtype=np.float32) * 100 + 4
    bt = np.concatenate([xy, xy + wh], 1)
    ps = rng.random((NS, C), dtype=np.float32) + 0.01
    ps /= ps.sum(1, keepdims=True)
    pt = rng.random((NT, C), dtype=np.float32) + 0.01
    pt /= pt.sum(1, keepdims=True)
    print("out:", kernel(bs, bt, ps, pt))
